# revision 11
# baseline (speedup 1.0000x reference)
"""MAGNN intra-metapath aggregator on 8 TRN2 NeuronCores — bf16 stream.

The kernel is HBM-bandwidth bound: it must stream all of `paths`
(819 MB fp32) once.  Streaming it as bf16 instead halves the bytes and
the roofline (288 us -> ~144 us); end-to-end quantization error is
~3e-4 (well under the 2e-2 gate).  The host casts/pads the tensor; all
actual computation (L-reduction, scores, leaky-relu softmax, weighted
accumulation) stays on device.

Per-core stream is padded 12500 -> 12544 = 24*512 + 256 instances so
every tile is uniform (no tail path).  Pad instances are all-zero, so
their reps contribution is 0 and their softmax weight is the known
constant exp(leakyrelu(b_h)) — subtracted exactly on the host.

Device pipeline per 512-instance tile (DMA budget 5.6 us @ 358 GB/s):
  DMA   2 MB, 4 instances/partition as one contiguous 16 KB line
  DVE   two bf16 tree-add passes 16 -> 4 node-blocks  (~3.5 us)
  PE    16 bf16 transposes (1 cyc/row) accumulate 4 blocks -> repsT,
        one 512-wide score matmul eT = a_rT.T @ rT, 4 w-transposes,
        4+4 accumulation matmuls into persistent PSUM      (~2.4 us)
  Act   PSUM->SBUF casts and w = max(exp(e+b), exp(0.2e+0.2b))
        (leaky-relu folded into two biased exp passes)     (~2.2 us)
"""

import numpy as np
import ml_dtypes

from concourse import bacc, masks, mybir, tile
from concourse.bass_utils import run_bass_kernel_spmd

N, L, D, H = 100000, 16, 128, 8
NCORES = 8
NS = N // NCORES            # 12500 real instances per core
NSP = 12544                 # padded: 24*512 + 256
NPAD = NSP - NS             # 44 zero instances per core
FD = L * D                  # 2048 elements per instance
F32 = mybir.dt.float32
BF16 = mybir.dt.bfloat16
AF = mybir.ActivationFunctionType

# (instances, lanes-per-partition) per streaming tile
BIGS = [(512, 4)] * 24 + [(256, 2)]

_cached_nc = None


def _build(repeat=1, **_compat):
    nc = bacc.Bacc(
        "TRN2",
        target_bir_lowering=False,
        debug=False,
        enable_asserts=False,
        num_devices=NCORES,
    )
    paths_d = nc.dram_tensor("paths", [NSP, L, D], BF16, kind="ExternalInput")
    tgt_d = nc.dram_tensor("target_feat", [D], F32, kind="ExternalInput")
    af_d = nc.dram_tensor("attn_fc", [H, 2 * D], F32, kind="ExternalInput")
    out_d = nc.dram_tensor("out", [H * (D + 1)], F32, kind="ExternalOutput")

    with tile.TileContext(nc) as tc:
        with (
            tc.tile_pool(name="const", bufs=1) as constp,
            tc.tile_pool(name="inp", bufs=3) as inp,
            tc.tile_pool(name="work", bufs=2) as work,
            tc.tile_pool(name="ps", bufs=1, space="PSUM") as psp,
        ):
            # ---------- constants ----------
            ident = constp.tile([128, 128], F32)
            masks.make_identity(nc, ident[:])
            identb = constp.tile([128, 128], BF16)
            masks.make_identity(nc, identb[:])
            af = constp.tile([H, 2 * D], F32)
            nc.sync.dma_start(af[:], af_d.ap())
            tf = constp.tile([D, 1], F32)
            nc.sync.dma_start(tf[:], tgt_d.ap().rearrange("(d one) -> d one", one=1))
            ones_col = constp.tile([128, 1], BF16)
            nc.vector.memset(ones_col[:], 1.0)

            # a_rT [D, H] bf16, scaled 1/L (folds the path-mean into scores)
            # (setup PSUM tiles reuse the main-loop rotating tags — PSUM
            # accumulation groups must never share a bank: start=True marks
            # the whole 2KB bank pending-zero)
            ps_r = psp.tile([128, 128], F32, tag="pt", bufs=1)
            nc.tensor.transpose(ps_r[:D, :H], af[:H, D : 2 * D], ident[:H, :H])
            a_rT = constp.tile([D, H], BF16)
            nc.scalar.mul(a_rT[:], ps_r[:D, :H], 1.0 / L)
            # a_tT [D, H] fp32 for the bias matmul
            ps_t = psp.tile([128, 128], F32, tag="pt", bufs=1)
            nc.tensor.transpose(ps_t[:D, :H], af[:H, 0:D], ident[:H, :H])
            a_tT = constp.tile([D, H], F32)
            nc.vector.tensor_copy(a_tT[:], ps_t[:D, :H])
            # per-head bias column b[h] = a_t[h] . target  (and 0.2*b)
            ps_b = psp.tile([128, 128], F32, tag="pt", bufs=1)
            nc.tensor.matmul(ps_b[:H, :1], a_tT[:, :H], tf[:, :1])
            b_col = constp.tile([H, 1], F32)
            nc.vector.tensor_copy(b_col[:], ps_b[:H, :1])
            b02_col = constp.tile([H, 1], F32)
            nc.scalar.mul(b02_col[:], ps_b[:H, :1], 0.2)
            # warm the exp table set early so the ~2.7us ACT_TABLE_LOAD
            # overlaps the first big DMA instead of stalling compute
            expwarm = constp.tile([H, 1], F32)
            nc.scalar.activation(expwarm[:], b_col[:], AF.Exp)

            # ---------- persistent accumulators ----------
            acc_p = psp.tile([H, 4 * D], F32, tag="accP")  # sum_n w * block_j
            acc_s = psp.tile([H, 1], F32, tag="accS")      # sum_n w

            paths2d = paths_d.ap().rearrange("n l d -> n (l d)")
            started = [False]
            total_bigs = repeat * len(BIGS)
            big_idx = [0]

            def do_big(n0, nb):
                first = not started[0]
                started[0] = True
                last = big_idx[0] == total_bigs - 1
                big_idx[0] += 1
                cnt = 128 * nb

                t = inp.tile([128, nb * FD], BF16, tag="in",
                             padded_shape=[128, 4 * FD])
                # partition p <- instances n0+nb*p+c, one contiguous line
                nc.sync.dma_start(
                    t[:],
                    paths2d[n0 : n0 + cnt, :].rearrange("(p b) f -> p (b f)", b=nb),
                )
                t3 = t.rearrange("p (b f) -> p b f", b=nb)
                # DVE tree: 16 -> 8 -> 4 node-blocks (bf16, 2 elem/cyc)
                t1 = work.tile([128, nb * 1024], BF16, tag="t1",
                               padded_shape=[128, 4 * 1024])
                t13 = t1.rearrange("p (b f) -> p b f", b=nb)
                nc.vector.tensor_add(t13[:, :, :], t3[:, :, 0:1024], t3[:, :, 1024:2048])
                t2 = work.tile([128, nb * 512], BF16, tag="t2",
                               padded_shape=[128, 4 * 512])
                t23 = t2.rearrange("p (b f) -> p b f", b=nb)
                nc.vector.tensor_add(t23[:, :, :], t13[:, :, 0:512], t13[:, :, 512:1024])

                # PE transposes of all 4 node-blocks -> [D, cnt] each.
                # 16-bit PSUM writes cannot accumulate on TRN2 (TRN3+ only),
                # so every transpose is an independent single write; the
                # 4-block fold happens in the fp32-accumulating e-matmuls.
                pt = psp.tile([128, nb * 512], BF16, tag="pt", bufs=1,
                              padded_shape=[128, 2048])
                for j in range(4):
                    for c in range(nb):
                        nc.tensor.matmul(
                            pt[:, (j * nb + c) * 128 : (j * nb + c + 1) * 128],
                            t2[:, (c * 4 + j) * 128 : (c * 4 + j + 1) * 128],
                            identb[:, :],
                            is_transpose=True,
                            start=True, stop=True,
                            skip_group_check=True,
                        )
                rT = work.tile([128, nb * 512], BF16, tag="rT",
                               padded_shape=[128, 2048])
                nc.scalar.copy(rT[:], pt[:])
                # scores eT[h, n] (pre-bias): accumulate the 4 block-slabs;
                # a_rT stays stationary across all four matmuls
                eT = psp.tile([H, nb * 128], F32, tag="eT", bufs=2,
                              padded_shape=[H, 512])
                for j in range(4):
                    nc.tensor.matmul(
                        eT[:, :], a_rT[:, :],
                        rT[:, j * nb * 128 : (j + 1) * nb * 128],
                        start=(j == 0), stop=(j == 3),
                    )
                # w = max(exp(e + b), exp(0.2e + 0.2b)) = exp(leakyrelu(e + b))
                wa = work.tile([H, nb * 128], BF16, tag="wa",
                               padded_shape=[H, 512])
                nc.scalar.activation(wa[:], eT[:, :], AF.Exp, bias=b_col[:], scale=1.0)
                wb = work.tile([H, nb * 128], BF16, tag="wb",
                               padded_shape=[H, 512])
                nc.scalar.activation(wb[:], eT[:, :], AF.Exp, bias=b02_col[:], scale=0.2)
                w2 = work.tile([H, nb * 128], BF16, tag="w2",
                               padded_shape=[H, 512])
                nc.vector.tensor_max(w2[:], wa[:], wb[:])
                # transpose w back to instance-layout [cnt, H]
                # (independent single writes into distinct slices)
                wps = psp.tile([128, nb * H], BF16, tag="wps", bufs=2,
                               padded_shape=[128, 4 * H])
                for c in range(nb):
                    nc.tensor.matmul(
                        wps[:, c * H : (c + 1) * H],
                        w2[:H, c * 128 : (c + 1) * 128],
                        identb[:H, :H],
                        is_transpose=True,
                        start=True, stop=True,
                        skip_group_check=True,
                    )
                w_sb = work.tile([128, nb * H], BF16, tag="w",
                                 padded_shape=[128, 4 * H])
                nc.scalar.copy(w_sb[:], wps[:])
                # accumulate sum_n w*block_j and sum_n w
                for c in range(nb):
                    bfirst = first and c == 0
                    blast = last and c == nb - 1
                    nc.tensor.matmul(
                        acc_p[:H, :],
                        w_sb[:, c * H : (c + 1) * H],
                        t2[:, c * 512 : (c + 1) * 512],
                        start=bfirst, stop=blast,
                    )
                    nc.tensor.matmul(
                        acc_s[:H, :],
                        w_sb[:, c * H : (c + 1) * H],
                        ones_col[:, :],
                        start=bfirst, stop=blast,
                    )

            # repeat>1 is a timing-only mode (re-streams the same shard)
            for _r in range(repeat):
                n0 = 0
                for cnt, nb in BIGS:
                    do_big(n0, nb)
                    n0 += cnt

            # ---------- emit per-core partial [p_raw | s] ----------
            # cross-core combine + softmax normalization happens on host
            accs = work.tile([H, 4 * D], F32, tag="accs")
            nc.vector.tensor_copy(accs[:H, :], acc_p[:H, :])
            fold = work.tile([H, 2 * D], F32, tag="fold")
            nc.vector.tensor_add(fold[:H, :], accs[:H, 0 : 2 * D], accs[:H, 2 * D : 4 * D])
            part = work.tile([H, D + 1], F32, tag="part")
            nc.vector.tensor_add(part[:H, 0:D], fold[:H, 0:D], fold[:H, D : 2 * D])
            nc.vector.tensor_copy(part[:H, D : D + 1], acc_s[:H, :])
            nc.sync.dma_start(
                out_d.ap().rearrange("(h d) -> h d", d=D + 1), part[:]
            )

    nc.compile()
    return nc


def _make_in_maps(target_feat, paths, attn_fc):
    """Shard + zero-pad to NSP and cast the streamed tensor to bf16."""
    tgt = np.ascontiguousarray(np.asarray(target_feat, dtype=np.float32))
    af = np.ascontiguousarray(np.asarray(attn_fc, dtype=np.float32))
    shards = np.asarray(paths, dtype=np.float32).reshape(NCORES, NS, L, D)
    padded = np.zeros((NCORES, NSP, L, D), dtype=ml_dtypes.bfloat16)
    padded[:, :NS] = shards.astype(ml_dtypes.bfloat16)
    return [
        {"paths": padded[i], "target_feat": tgt, "attn_fc": af}
        for i in range(NCORES)
    ]


def kernel(target_feat, paths, attn_fc, **_unused):
    global _cached_nc
    if _cached_nc is None:
        _cached_nc = _build()
    nc = _cached_nc

    in_maps = _make_in_maps(target_feat, paths, attn_fc)
    res = run_bass_kernel_spmd(nc, in_maps, core_ids=list(range(NCORES)))
    # host-side combine of the 8 per-core partials [H, D+1]
    tot = np.zeros((H, D + 1), dtype=np.float64)
    for i in range(NCORES):
        tot += np.asarray(res.results[i]["out"], dtype=np.float64).reshape(
            H, D + 1
        )
    # subtract the pad instances' exact softmax-weight contribution:
    # each zero instance has reps=0 -> w = exp(leakyrelu(b_h))
    b = attn_fc[:, :D].astype(np.float64) @ np.asarray(target_feat, np.float64)
    w_pad = np.exp(np.where(b > 0, b, 0.2 * b))
    s = tot[:, D] - NCORES * NPAD * w_pad
    out = tot[:, :D] / (L * s[:, None])
    return np.ascontiguousarray(out.reshape(H * D).astype(np.float32))


# revision 15
# speedup vs baseline: 1.5433x; 1.5433x over previous
"""MAGNN intra-metapath aggregator on 8 TRN2 NeuronCores — bf16 stream.

The kernel is HBM-bandwidth bound: it must stream all of `paths`
(819 MB fp32) once.  Streaming it as bf16 instead halves the bytes and
the roofline (288 us -> ~144 us); end-to-end quantization error is
~3e-4 (well under the 2e-2 gate).  The host casts/pads the tensor; all
actual computation (L-reduction, scores, leaky-relu softmax, weighted
accumulation) stays on device.

Per-core stream is padded 12500 -> 12544 = 24*512 + 256 instances so
every tile is uniform (no tail path).  Pad instances are all-zero, so
their reps contribution is 0 and their softmax weight is the known
constant exp(leakyrelu(b_h)) — subtracted exactly on the host.

Device pipeline per 512-instance tile (DMA budget 5.6 us @ 358 GB/s):
  DMA   2 MB, 4 instances/partition as one contiguous 16 KB line
  DVE   two bf16 tree-add passes 16 -> 4 node-blocks  (~3.5 us)
  PE    16 bf16 transposes (1 cyc/row) accumulate 4 blocks -> repsT,
        one 512-wide score matmul eT = a_rT.T @ rT, 4 w-transposes,
        4+4 accumulation matmuls into persistent PSUM      (~2.4 us)
  Act   PSUM->SBUF casts and w = max(exp(e+b), exp(0.2e+0.2b))
        (leaky-relu folded into two biased exp passes)     (~2.2 us)
"""

import numpy as np
import ml_dtypes

from concourse import bacc, masks, mybir, tile
from concourse.bass_utils import run_bass_kernel_spmd

N, L, D, H = 100000, 16, 128, 8
NCORES = 8
NS = N // NCORES            # 12500 real instances per core
NSP = 12544                 # padded: 24*512 + 256
NPAD = NSP - NS             # 44 zero instances per core
FD = L * D                  # 2048 elements per instance
F32 = mybir.dt.float32
BF16 = mybir.dt.bfloat16
AF = mybir.ActivationFunctionType

# (instances, lanes-per-partition) per streaming tile
BIGS = [(512, 4)] * 24 + [(256, 2)]

_cached_nc = None


def _build(repeat=1, inp_bufs=3, work_bufs=2, pt_bufs=1, eT_bufs=2,
           wps_bufs=2, **_compat):
    nc = bacc.Bacc(
        "TRN2",
        target_bir_lowering=False,
        debug=False,
        enable_asserts=False,
        num_devices=NCORES,
    )
    paths_d = nc.dram_tensor("paths", [NSP, L, D], BF16, kind="ExternalInput")
    tgt_d = nc.dram_tensor("target_feat", [D], F32, kind="ExternalInput")
    af_d = nc.dram_tensor("attn_fc", [H, 2 * D], F32, kind="ExternalInput")
    out_d = nc.dram_tensor("out", [H * (D + 1)], F32, kind="ExternalOutput")

    with tile.TileContext(nc) as tc:
        with (
            tc.tile_pool(name="const", bufs=1) as constp,
            tc.tile_pool(name="inp", bufs=inp_bufs) as inp,
            tc.tile_pool(name="work", bufs=work_bufs) as work,
            tc.tile_pool(name="ps", bufs=1, space="PSUM") as psp,
        ):
            # ---------- constants ----------
            ident = constp.tile([128, 128], F32)
            masks.make_identity(nc, ident[:])
            identb = constp.tile([128, 128], BF16)
            masks.make_identity(nc, identb[:])
            ones_col = constp.tile([128, 1], BF16)
            nc.vector.memset(ones_col[:], 1.0)
            # warm the exp table set first so the ~2.7us ACT_TABLE_LOAD
            # overlaps the first big DMA instead of stalling the first
            # Act op (even scalar.copy needs a loaded set)
            expwarm = constp.tile([H, 1], F32)
            nc.scalar.activation(expwarm[:], ident[:H, :1], AF.Exp)

            af = constp.tile([H, 2 * D], F32)
            tf = constp.tile([D, 1], F32)
            a_rT = constp.tile([D, H], BF16)
            a_tT = constp.tile([D, H], F32)
            b_col = constp.tile([H, 1], F32)
            b02_col = constp.tile([H, 1], F32)

            def setup_consts():
                # emitted AFTER head(0) so the tiny af/tf DMAs queue behind
                # the first 2MB stream instead of delaying it
                nc.sync.dma_start(af[:], af_d.ap())
                nc.sync.dma_start(
                    tf[:], tgt_d.ap().rearrange("(d one) -> d one", one=1))
                # a_rT [D, H] bf16, scaled 1/L (folds the path-mean into
                # scores).  Setup PSUM tiles reuse the rotating "pt" tag —
                # PSUM accumulation groups must never share a bank
                # (start=True marks the whole 2KB bank pending-zero).
                ps_r = psp.tile([128, 128], F32, tag="pt", bufs=pt_bufs)
                nc.tensor.transpose(ps_r[:D, :H], af[:H, D : 2 * D], ident[:H, :H])
                nc.scalar.mul(a_rT[:], ps_r[:D, :H], 1.0 / L)
                # a_tT [D, H] fp32 for the bias matmul
                ps_t = psp.tile([128, 128], F32, tag="pt", bufs=pt_bufs)
                nc.tensor.transpose(ps_t[:D, :H], af[:H, 0:D], ident[:H, :H])
                nc.vector.tensor_copy(a_tT[:], ps_t[:D, :H])
                # per-head bias column b[h] = a_t[h] . target  (and 0.2*b)
                ps_b = psp.tile([128, 128], F32, tag="pt", bufs=pt_bufs)
                nc.tensor.matmul(ps_b[:H, :1], a_tT[:, :H], tf[:, :1])
                nc.vector.tensor_copy(b_col[:], ps_b[:H, :1])
                nc.scalar.mul(b02_col[:], ps_b[:H, :1], 0.2)

            # ---------- persistent accumulators ----------
            acc_p = psp.tile([H, 4 * D], F32, tag="accP")  # sum_n w * block_j
            acc_s = psp.tile([H, 1], F32, tag="accS")      # sum_n w

            paths2d = paths_d.ap().rearrange("n l d -> n (l d)")

            # The per-tile work is split into head (DMA, DVE tree, PE
            # transposes, Act rT-copy) and tail (score matmuls, exps, max,
            # w-transposes, accumulation).  head(i) is emitted BEFORE
            # tail(i-1): engines execute their queues in strict FIFO order,
            # so emitting a tile's whole chain contiguously would chain the
            # next tile's rT-copy behind this tile's w-copy — a serial
            # cycle of ~6.9 us > the 5.8 us DMA period.  The head/tail
            # interleave keeps every engine's FIFO one iteration deep.
            def head(n0, nb):
                cnt = 128 * nb
                t = inp.tile([128, nb * FD], BF16, tag="in",
                             padded_shape=[128, 4 * FD])
                # partition p <- instances n0+nb*p+c, one contiguous line
                nc.sync.dma_start(
                    t[:],
                    paths2d[n0 : n0 + cnt, :].rearrange("(p b) f -> p (b f)", b=nb),
                )
                t3 = t.rearrange("p (b f) -> p b f", b=nb)
                # DVE tree: 16 -> 8 -> 4 node-blocks (bf16, 2 elem/cyc)
                t1 = work.tile([128, nb * 1024], BF16, tag="t1",
                               padded_shape=[128, 4 * 1024])
                t13 = t1.rearrange("p (b f) -> p b f", b=nb)
                nc.vector.tensor_add(t13[:, :, :], t3[:, :, 0:1024], t3[:, :, 1024:2048])
                t2 = work.tile([128, nb * 512], BF16, tag="t2", bufs=3,
                               padded_shape=[128, 4 * 512])
                t23 = t2.rearrange("p (b f) -> p b f", b=nb)
                nc.vector.tensor_add(t23[:, :, :], t13[:, :, 0:512], t13[:, :, 512:1024])

                # PE transposes of all 4 node-blocks -> [D, cnt] each.
                # 16-bit PSUM writes cannot accumulate on TRN2 (TRN3+ only),
                # so every transpose is an independent single write; the
                # 4-block fold happens in the fp32-accumulating e-matmuls.
                pt = psp.tile([128, nb * 512], BF16, tag="pt", bufs=pt_bufs,
                              padded_shape=[128, 2048])
                for j in range(4):
                    for c in range(nb):
                        nc.tensor.matmul(
                            pt[:, (j * nb + c) * 128 : (j * nb + c + 1) * 128],
                            t2[:, (c * 4 + j) * 128 : (c * 4 + j + 1) * 128],
                            identb[:, :],
                            is_transpose=True,
                            start=True, stop=True,
                            skip_group_check=True,
                        )
                rT = work.tile([128, nb * 512], BF16, tag="rT", bufs=3,
                               padded_shape=[128, 2048])
                nc.scalar.copy(rT[:], pt[:])
                return t2, rT, nb

            def tail(state, first, last):
                t2, rT, nb = state
                # scores eT[h, n] (pre-bias): accumulate the 4 block-slabs;
                # a_rT stays stationary across all four matmuls
                eT = psp.tile([H, nb * 128], F32, tag="eT", bufs=eT_bufs,
                              padded_shape=[H, 512])
                for j in range(4):
                    nc.tensor.matmul(
                        eT[:, :], a_rT[:, :],
                        rT[:, j * nb * 128 : (j + 1) * nb * 128],
                        start=(j == 0), stop=(j == 3),
                    )
                # w = max(exp(e + b), exp(0.2e + 0.2b)) = exp(leakyrelu(e + b))
                wa = work.tile([H, nb * 128], BF16, tag="wa",
                               padded_shape=[H, 512])
                nc.scalar.activation(wa[:], eT[:, :], AF.Exp, bias=b_col[:], scale=1.0)
                wb = work.tile([H, nb * 128], BF16, tag="wb",
                               padded_shape=[H, 512])
                nc.scalar.activation(wb[:], eT[:, :], AF.Exp, bias=b02_col[:], scale=0.2)
                w2 = work.tile([H, nb * 128], BF16, tag="w2",
                               padded_shape=[H, 512])
                nc.vector.tensor_max(w2[:], wa[:], wb[:])
                # transpose w back to instance-layout [cnt, H]
                # (independent single writes into distinct slices)
                wps = psp.tile([128, nb * H], BF16, tag="wps", bufs=wps_bufs,
                               padded_shape=[128, 4 * H])
                for c in range(nb):
                    nc.tensor.matmul(
                        wps[:, c * H : (c + 1) * H],
                        w2[:H, c * 128 : (c + 1) * 128],
                        identb[:H, :H],
                        is_transpose=True,
                        start=True, stop=True,
                        skip_group_check=True,
                    )
                w_sb = work.tile([128, nb * H], BF16, tag="w",
                                 padded_shape=[128, 4 * H])
                nc.scalar.copy(w_sb[:], wps[:])
                # accumulate sum_n w*block_j and sum_n w
                for c in range(nb):
                    bfirst = first and c == 0
                    blast = last and c == nb - 1
                    nc.tensor.matmul(
                        acc_p[:H, :],
                        w_sb[:, c * H : (c + 1) * H],
                        t2[:, c * 512 : (c + 1) * 512],
                        start=bfirst, stop=blast,
                    )
                    nc.tensor.matmul(
                        acc_s[:H, :],
                        w_sb[:, c * H : (c + 1) * H],
                        ones_col[:, :],
                        start=bfirst, stop=blast,
                    )

            # repeat>1 is a timing-only mode (re-streams the same shard)
            seq = []
            for _r in range(repeat):
                n0 = 0
                for cnt, nb in BIGS:
                    seq.append((n0, nb))
                    n0 += cnt
            pending = None
            for i, (n0, nb) in enumerate(seq):
                st = head(n0, nb)
                if i == 0:
                    setup_consts()
                if pending is not None:
                    tail(pending, first=(i == 1), last=False)
                pending = st
            tail(pending, first=(len(seq) == 1), last=True)

            # ---------- emit per-core partial [p_raw | s] ----------
            # cross-core combine + softmax normalization happens on host
            accs = work.tile([H, 4 * D], F32, tag="accs")
            nc.vector.tensor_copy(accs[:H, :], acc_p[:H, :])
            fold = work.tile([H, 2 * D], F32, tag="fold")
            nc.vector.tensor_add(fold[:H, :], accs[:H, 0 : 2 * D], accs[:H, 2 * D : 4 * D])
            part = work.tile([H, D + 1], F32, tag="part")
            nc.vector.tensor_add(part[:H, 0:D], fold[:H, 0:D], fold[:H, D : 2 * D])
            nc.vector.tensor_copy(part[:H, D : D + 1], acc_s[:H, :])
            nc.sync.dma_start(
                out_d.ap().rearrange("(h d) -> h d", d=D + 1), part[:]
            )

    nc.compile()
    return nc


def _make_in_maps(target_feat, paths, attn_fc):
    """Shard + zero-pad to NSP and cast the streamed tensor to bf16."""
    tgt = np.ascontiguousarray(np.asarray(target_feat, dtype=np.float32))
    af = np.ascontiguousarray(np.asarray(attn_fc, dtype=np.float32))
    shards = np.asarray(paths, dtype=np.float32).reshape(NCORES, NS, L, D)
    padded = np.zeros((NCORES, NSP, L, D), dtype=ml_dtypes.bfloat16)
    padded[:, :NS] = shards.astype(ml_dtypes.bfloat16)
    return [
        {"paths": padded[i], "target_feat": tgt, "attn_fc": af}
        for i in range(NCORES)
    ]


def kernel(target_feat, paths, attn_fc, **_unused):
    global _cached_nc
    if _cached_nc is None:
        _cached_nc = _build()
    nc = _cached_nc

    in_maps = _make_in_maps(target_feat, paths, attn_fc)
    res = run_bass_kernel_spmd(nc, in_maps, core_ids=list(range(NCORES)))
    # host-side combine of the 8 per-core partials [H, D+1]
    tot = np.zeros((H, D + 1), dtype=np.float64)
    for i in range(NCORES):
        tot += np.asarray(res.results[i]["out"], dtype=np.float64).reshape(
            H, D + 1
        )
    # subtract the pad instances' exact softmax-weight contribution:
    # each zero instance has reps=0 -> w = exp(leakyrelu(b_h))
    b = attn_fc[:, :D].astype(np.float64) @ np.asarray(target_feat, np.float64)
    w_pad = np.exp(np.where(b > 0, b, 0.2 * b))
    s = tot[:, D] - NCORES * NPAD * w_pad
    out = tot[:, :D] / (L * s[:, None])
    return np.ascontiguousarray(out.reshape(H * D).astype(np.float32))


# revision 17
# speedup vs baseline: 1.7115x; 1.1090x over previous
"""MAGNN intra-metapath aggregator on 8 TRN2 NeuronCores — bf16 stream.

The kernel is HBM-bandwidth bound: it must stream all of `paths`
(819 MB fp32) once.  Streaming it as bf16 halves the bytes and the
roofline; end-to-end quantization error is ~1.4e-3 (gate: 2e-2).  The
host only casts/pads the tensor; all actual computation (L-reduction,
scores, leaky-relu softmax, weighted accumulation) stays on device.
Measured: 288 us (fp32 baseline) -> ~120 us.

Per-core stream is padded 12500 -> 12544 = 24*512 + 256 instances so
every tile is uniform (no tail path).  Pad instances are all-zero, so
their reps contribution is 0 and their softmax weight is the known
constant exp(leakyrelu(b_h)) — subtracted exactly on the host.

Device pipeline per 512-instance tile (DMA ~4.1-5.8 us / 2 MB):
  DMA   2 MB, 4 instances/partition as one contiguous 16 KB line
  DVE   two bf16 tree-add passes 16 -> 4 node-blocks     (~3.7 us)
  PE    16 bf16 transposes (no PSUM accumulate: 16-bit PSUM writes
        can't accumulate on TRN2) -> 4 repsT slabs; 4 fp32-accumulating
        score matmuls eT = sum_j a_rT.T @ rT_j; 4 w-transposes;
        4+4 accumulation matmuls into persistent PSUM     (~2.7 us)
  Act   PSUM->SBUF casts and w = max(exp(e+b), exp(0.2e+0.2b))
        (leaky-relu folded into two biased exp passes)    (~3.4 us)

The tile loop is software-pipelined: head(i) = DMA/tree/transposes/
rT-copy is emitted before tail(i-1) = scores/softmax/accumulate, so no
strict-FIFO engine queue bridges a long dependency chain (that cycle
otherwise sets a ~6.9 us period vs the 5.8 us DMA period).
"""

import numpy as np
import ml_dtypes

from concourse import bacc, masks, mybir, tile
from concourse.bass_utils import run_bass_kernel_spmd

N, L, D, H = 100000, 16, 128, 8
NCORES = 8
NS = N // NCORES            # 12500 real instances per core
NSP = 12544                 # padded: 24*512 + 256
NPAD = NSP - NS             # 44 zero instances per core
FD = L * D                  # 2048 elements per instance
F32 = mybir.dt.float32
BF16 = mybir.dt.bfloat16
AF = mybir.ActivationFunctionType

# (instances, lanes-per-partition) per streaming tile
BIGS = [(512, 4)] * 24 + [(256, 2)]

_cached_nc = None


def _build(repeat=1, inp_bufs=3, work_bufs=2, pt_bufs=1, eT_bufs=2,
           wps_bufs=2, **_compat):
    nc = bacc.Bacc(
        "TRN2",
        target_bir_lowering=False,
        debug=False,
        enable_asserts=False,
        num_devices=NCORES,
    )
    paths_d = nc.dram_tensor("paths", [NSP, L, D], BF16, kind="ExternalInput")
    tgt_d = nc.dram_tensor("target_feat", [D], F32, kind="ExternalInput")
    af_d = nc.dram_tensor("attn_fc", [H, 2 * D], F32, kind="ExternalInput")
    out_d = nc.dram_tensor("out", [H * (D + 1)], F32, kind="ExternalOutput")

    with tile.TileContext(nc) as tc:
        with (
            tc.tile_pool(name="const", bufs=1) as constp,
            tc.tile_pool(name="inp", bufs=inp_bufs) as inp,
            tc.tile_pool(name="work", bufs=work_bufs) as work,
            tc.tile_pool(name="ps", bufs=1, space="PSUM") as psp,
        ):
            # ---------- constants ----------
            ident = constp.tile([128, 128], F32)
            masks.make_identity(nc, ident[:])
            identb = constp.tile([128, 128], BF16)
            masks.make_identity(nc, identb[:])
            ones_col = constp.tile([128, 1], BF16)
            nc.vector.memset(ones_col[:], 1.0)
            # warm the exp table set first so the ~2.7us ACT_TABLE_LOAD
            # overlaps the first big DMA instead of stalling the first
            # Act op (even scalar.copy needs a loaded set)
            expwarm = constp.tile([H, 1], F32)
            nc.scalar.activation(expwarm[:], ident[:H, :1], AF.Exp)

            af = constp.tile([H, 2 * D], F32)
            tf = constp.tile([D, 1], F32)
            a_rT = constp.tile([D, H], BF16)
            a_tT = constp.tile([D, H], F32)
            b_col = constp.tile([H, 1], F32)
            b02_col = constp.tile([H, 1], F32)

            def setup_consts():
                # emitted AFTER head(0) so the tiny af/tf DMAs queue behind
                # the first 2MB stream instead of delaying it
                nc.sync.dma_start(af[:], af_d.ap())
                nc.sync.dma_start(
                    tf[:], tgt_d.ap().rearrange("(d one) -> d one", one=1))
                # a_rT [D, H] bf16, scaled 1/L (folds the path-mean into
                # scores).  Setup PSUM tiles reuse the rotating "pt" tag —
                # PSUM accumulation groups must never share a bank
                # (start=True marks the whole 2KB bank pending-zero).
                ps_r = psp.tile([128, 128], F32, tag="pt", bufs=pt_bufs)
                nc.tensor.transpose(ps_r[:D, :H], af[:H, D : 2 * D], ident[:H, :H])
                nc.scalar.mul(a_rT[:], ps_r[:D, :H], 1.0 / L)
                # a_tT [D, H] fp32 for the bias matmul
                ps_t = psp.tile([128, 128], F32, tag="pt", bufs=pt_bufs)
                nc.tensor.transpose(ps_t[:D, :H], af[:H, 0:D], ident[:H, :H])
                nc.vector.tensor_copy(a_tT[:], ps_t[:D, :H])
                # per-head bias column b[h] = a_t[h] . target  (and 0.2*b)
                ps_b = psp.tile([128, 128], F32, tag="pt", bufs=pt_bufs)
                nc.tensor.matmul(ps_b[:H, :1], a_tT[:, :H], tf[:, :1])
                nc.vector.tensor_copy(b_col[:], ps_b[:H, :1])
                nc.scalar.mul(b02_col[:], ps_b[:H, :1], 0.2)

            # ---------- persistent accumulators ----------
            acc_p = psp.tile([H, 4 * D], F32, tag="accP")  # sum_n w * block_j
            acc_s = psp.tile([H, 1], F32, tag="accS")      # sum_n w

            paths2d = paths_d.ap().rearrange("n l d -> n (l d)")

            # The per-tile work is split into head (DMA, DVE tree, PE
            # transposes, Act rT-copy) and tail (score matmuls, exps, max,
            # w-transposes, accumulation).  head(i) is emitted BEFORE
            # tail(i-1): engines execute their queues in strict FIFO order,
            # so emitting a tile's whole chain contiguously would chain the
            # next tile's rT-copy behind this tile's w-copy — a serial
            # cycle of ~6.9 us > the 5.8 us DMA period.  The head/tail
            # interleave keeps every engine's FIFO one iteration deep.
            def head(n0, nb):
                cnt = 128 * nb
                t = inp.tile([128, nb * FD], BF16, tag="in",
                             padded_shape=[128, 4 * FD])
                # partition p <- instances n0+nb*p+c, one contiguous line
                nc.sync.dma_start(
                    t[:],
                    paths2d[n0 : n0 + cnt, :].rearrange("(p b) f -> p (b f)", b=nb),
                )
                t3 = t.rearrange("p (b f) -> p b f", b=nb)
                # DVE tree: 16 -> 8 -> 4 node-blocks (bf16, 2 elem/cyc)
                t1 = work.tile([128, nb * 1024], BF16, tag="t1",
                               padded_shape=[128, 4 * 1024])
                t13 = t1.rearrange("p (b f) -> p b f", b=nb)
                nc.vector.tensor_add(t13[:, :, :], t3[:, :, 0:1024], t3[:, :, 1024:2048])
                t2 = work.tile([128, nb * 512], BF16, tag="t2", bufs=3,
                               padded_shape=[128, 4 * 512])
                t23 = t2.rearrange("p (b f) -> p b f", b=nb)
                nc.vector.tensor_add(t23[:, :, :], t13[:, :, 0:512], t13[:, :, 512:1024])

                # PE transposes of all 4 node-blocks -> [D, cnt] each.
                # 16-bit PSUM writes cannot accumulate on TRN2 (TRN3+ only),
                # so every transpose is an independent single write; the
                # 4-block fold happens in the fp32-accumulating e-matmuls.
                pt = psp.tile([128, nb * 512], BF16, tag="pt", bufs=pt_bufs,
                              padded_shape=[128, 2048])
                for j in range(4):
                    for c in range(nb):
                        nc.tensor.matmul(
                            pt[:, (j * nb + c) * 128 : (j * nb + c + 1) * 128],
                            t2[:, (c * 4 + j) * 128 : (c * 4 + j + 1) * 128],
                            identb[:, :],
                            is_transpose=True,
                            start=True, stop=True,
                            skip_group_check=True,
                        )
                rT = work.tile([128, nb * 512], BF16, tag="rT", bufs=3,
                               padded_shape=[128, 2048])
                # split the copy so the first e-matmuls can start while the
                # second half is still copying
                half = nb * 256
                nc.scalar.copy(rT[:, 0:half], pt[:, 0:half])
                nc.scalar.copy(rT[:, half : 2 * half], pt[:, half : 2 * half])
                return t2, rT, nb

            def tail(state, first, last):
                t2, rT, nb = state
                # scores eT[h, n] (pre-bias): accumulate the 4 block-slabs;
                # a_rT stays stationary across all four matmuls
                eT = psp.tile([H, nb * 128], F32, tag="eT", bufs=eT_bufs,
                              padded_shape=[H, 512])
                for j in range(4):
                    nc.tensor.matmul(
                        eT[:, :], a_rT[:, :],
                        rT[:, j * nb * 128 : (j + 1) * nb * 128],
                        start=(j == 0), stop=(j == 3),
                    )
                # w = max(exp(e + b), exp(0.2e + 0.2b)) = exp(leakyrelu(e + b))
                wa = work.tile([H, nb * 128], BF16, tag="wa",
                               padded_shape=[H, 512])
                nc.scalar.activation(wa[:], eT[:, :], AF.Exp, bias=b_col[:], scale=1.0)
                wb = work.tile([H, nb * 128], BF16, tag="wb",
                               padded_shape=[H, 512])
                nc.scalar.activation(wb[:], eT[:, :], AF.Exp, bias=b02_col[:], scale=0.2)
                w2 = work.tile([H, nb * 128], BF16, tag="w2",
                               padded_shape=[H, 512])
                nc.vector.tensor_max(w2[:], wa[:], wb[:])
                # transpose w back to instance-layout [cnt, H]
                # (independent single writes into distinct slices)
                wps = psp.tile([128, nb * H], BF16, tag="wps", bufs=wps_bufs,
                               padded_shape=[128, 4 * H])
                for c in range(nb):
                    nc.tensor.matmul(
                        wps[:, c * H : (c + 1) * H],
                        w2[:H, c * 128 : (c + 1) * 128],
                        identb[:H, :H],
                        is_transpose=True,
                        start=True, stop=True,
                        skip_group_check=True,
                    )
                w_sb = work.tile([128, nb * H], BF16, tag="w",
                                 padded_shape=[128, 4 * H])
                nc.scalar.copy(w_sb[:], wps[:])
                # accumulate sum_n w*block_j and sum_n w
                for c in range(nb):
                    bfirst = first and c == 0
                    blast = last and c == nb - 1
                    nc.tensor.matmul(
                        acc_p[:H, :],
                        w_sb[:, c * H : (c + 1) * H],
                        t2[:, c * 512 : (c + 1) * 512],
                        start=bfirst, stop=blast,
                    )
                    nc.tensor.matmul(
                        acc_s[:H, :],
                        w_sb[:, c * H : (c + 1) * H],
                        ones_col[:, :],
                        start=bfirst, stop=blast,
                    )

            # repeat>1 is a timing-only mode (re-streams the same shard)
            seq = []
            for _r in range(repeat):
                n0 = 0
                for cnt, nb in BIGS:
                    seq.append((n0, nb))
                    n0 += cnt
            pending = None
            for i, (n0, nb) in enumerate(seq):
                st = head(n0, nb)
                if i == 0:
                    setup_consts()
                if pending is not None:
                    tail(pending, first=(i == 1), last=False)
                pending = st
            tail(pending, first=(len(seq) == 1), last=True)

            # ---------- emit per-core partial [p_raw | s] ----------
            # cross-core combine + softmax normalization happens on host
            accs = work.tile([H, 4 * D], F32, tag="accs")
            nc.vector.tensor_copy(accs[:H, :], acc_p[:H, :])
            fold = work.tile([H, 2 * D], F32, tag="fold")
            nc.vector.tensor_add(fold[:H, :], accs[:H, 0 : 2 * D], accs[:H, 2 * D : 4 * D])
            part = work.tile([H, D + 1], F32, tag="part")
            nc.vector.tensor_add(part[:H, 0:D], fold[:H, 0:D], fold[:H, D : 2 * D])
            nc.vector.tensor_copy(part[:H, D : D + 1], acc_s[:H, :])
            nc.sync.dma_start(
                out_d.ap().rearrange("(h d) -> h d", d=D + 1), part[:]
            )

    nc.compile()
    return nc


def _make_in_maps(target_feat, paths, attn_fc):
    """Shard + zero-pad to NSP and cast the streamed tensor to bf16."""
    tgt = np.ascontiguousarray(np.asarray(target_feat, dtype=np.float32))
    af = np.ascontiguousarray(np.asarray(attn_fc, dtype=np.float32))
    shards = np.asarray(paths, dtype=np.float32).reshape(NCORES, NS, L, D)
    padded = np.zeros((NCORES, NSP, L, D), dtype=ml_dtypes.bfloat16)
    padded[:, :NS] = shards.astype(ml_dtypes.bfloat16)
    return [
        {"paths": padded[i], "target_feat": tgt, "attn_fc": af}
        for i in range(NCORES)
    ]


def kernel(target_feat, paths, attn_fc, **_unused):
    global _cached_nc
    if _cached_nc is None:
        _cached_nc = _build()
    nc = _cached_nc

    in_maps = _make_in_maps(target_feat, paths, attn_fc)
    res = run_bass_kernel_spmd(nc, in_maps, core_ids=list(range(NCORES)))
    # host-side combine of the 8 per-core partials [H, D+1]
    tot = np.zeros((H, D + 1), dtype=np.float64)
    for i in range(NCORES):
        tot += np.asarray(res.results[i]["out"], dtype=np.float64).reshape(
            H, D + 1
        )
    # subtract the pad instances' exact softmax-weight contribution:
    # each zero instance has reps=0 -> w = exp(leakyrelu(b_h))
    b = attn_fc[:, :D].astype(np.float64) @ np.asarray(target_feat, np.float64)
    w_pad = np.exp(np.where(b > 0, b, 0.2 * b))
    s = tot[:, D] - NCORES * NPAD * w_pad
    out = tot[:, :D] / (L * s[:, None])
    return np.ascontiguousarray(out.reshape(H * D).astype(np.float32))


# revision 18
# speedup vs baseline: 1.7876x; 1.0445x over previous
"""MAGNN intra-metapath aggregator on 8 TRN2 NeuronCores — bf16 stream.

The kernel is HBM-bandwidth bound: it must stream all of `paths`
(819 MB fp32) once.  Streaming it as bf16 halves the bytes and the
roofline; end-to-end quantization error is ~1.4e-3 (gate: 2e-2).  The
host only casts/pads the tensor; all actual computation (L-reduction,
scores, leaky-relu softmax, weighted accumulation) stays on device.
Measured: 288 us (fp32 baseline) -> ~120 us.

Per-core stream is padded 12500 -> 12544 = 24*512 + 256 instances so
every tile is uniform (no tail path).  Pad instances are all-zero, so
their reps contribution is 0 and their softmax weight is the known
constant exp(leakyrelu(b_h)) — subtracted exactly on the host.

Device pipeline per 512-instance tile (DMA ~4.1-5.8 us / 2 MB):
  DMA   2 MB, 4 instances/partition as one contiguous 16 KB line
  DVE   two bf16 tree-add passes 16 -> 4 node-blocks     (~3.7 us)
  PE    16 bf16 transposes (no PSUM accumulate: 16-bit PSUM writes
        can't accumulate on TRN2) -> 4 repsT slabs; 4 fp32-accumulating
        score matmuls eT = sum_j a_rT.T @ rT_j; 4 w-transposes;
        4+4 accumulation matmuls into persistent PSUM     (~2.7 us)
  Act   PSUM->SBUF casts and w = max(exp(e+b), exp(0.2e+0.2b))
        (leaky-relu folded into two biased exp passes)    (~3.4 us)

The tile loop is software-pipelined: head(i) = DMA/tree/transposes/
rT-copy is emitted before tail(i-1) = scores/softmax/accumulate, so no
strict-FIFO engine queue bridges a long dependency chain (that cycle
otherwise sets a ~6.9 us period vs the 5.8 us DMA period).
"""

import numpy as np
import ml_dtypes

from concourse import bacc, masks, mybir, tile
from concourse.bass_utils import run_bass_kernel_spmd

N, L, D, H = 100000, 16, 128, 8
NCORES = 8
NS = N // NCORES            # 12500 real instances per core
NSP = 12544                 # padded: 24*512 + 256
NPAD = NSP - NS             # 44 zero instances per core
FD = L * D                  # 2048 elements per instance
F32 = mybir.dt.float32
BF16 = mybir.dt.bfloat16
AF = mybir.ActivationFunctionType

# (instances, lanes-per-partition) per streaming tile
BIGS = [(512, 4)] * 24 + [(256, 2)]

_cached_nc = None


def _build(repeat=1, inp_bufs=4, work_bufs=2, pt_bufs=1, eT_bufs=2,
           wps_bufs=2, **_compat):
    nc = bacc.Bacc(
        "TRN2",
        target_bir_lowering=False,
        debug=False,
        enable_asserts=False,
        num_devices=NCORES,
    )
    paths_d = nc.dram_tensor("paths", [NSP, L, D], BF16, kind="ExternalInput")
    tgt_d = nc.dram_tensor("target_feat", [D], F32, kind="ExternalInput")
    af_d = nc.dram_tensor("attn_fc", [H, 2 * D], F32, kind="ExternalInput")
    out_d = nc.dram_tensor("out", [H * (D + 1)], F32, kind="ExternalOutput")

    with tile.TileContext(nc) as tc:
        with (
            tc.tile_pool(name="const", bufs=1) as constp,
            tc.tile_pool(name="inp", bufs=inp_bufs) as inp,
            tc.tile_pool(name="work", bufs=work_bufs) as work,
            tc.tile_pool(name="ps", bufs=1, space="PSUM") as psp,
        ):
            # ---------- constants ----------
            ident = constp.tile([128, 128], F32)
            masks.make_identity(nc, ident[:])
            identb = constp.tile([128, 128], BF16)
            masks.make_identity(nc, identb[:])
            ones_col = constp.tile([128, 1], BF16)
            nc.vector.memset(ones_col[:], 1.0)
            # warm the exp table set first so the ~2.7us ACT_TABLE_LOAD
            # overlaps the first big DMA instead of stalling the first
            # Act op (even scalar.copy needs a loaded set)
            expwarm = constp.tile([H, 1], F32)
            nc.scalar.activation(expwarm[:], ident[:H, :1], AF.Exp)

            af = constp.tile([H, 2 * D], F32)
            tf = constp.tile([D, 1], F32)
            a_rT = constp.tile([D, H], BF16)
            a_tT = constp.tile([D, H], F32)
            b_col = constp.tile([H, 1], F32)
            b02_col = constp.tile([H, 1], F32)

            def setup_consts():
                # emitted AFTER head(0) so the tiny af/tf DMAs queue behind
                # the first 2MB stream instead of delaying it
                nc.sync.dma_start(af[:], af_d.ap())
                nc.sync.dma_start(
                    tf[:], tgt_d.ap().rearrange("(d one) -> d one", one=1))
                # a_rT [D, H] bf16, scaled 1/L (folds the path-mean into
                # scores).  Setup PSUM tiles reuse the rotating "pt" tag —
                # PSUM accumulation groups must never share a bank
                # (start=True marks the whole 2KB bank pending-zero).
                ps_r = psp.tile([128, 128], F32, tag="pt", bufs=pt_bufs)
                nc.tensor.transpose(ps_r[:D, :H], af[:H, D : 2 * D], ident[:H, :H])
                nc.scalar.mul(a_rT[:], ps_r[:D, :H], 1.0 / L)
                # a_tT [D, H] fp32 for the bias matmul
                ps_t = psp.tile([128, 128], F32, tag="pt", bufs=pt_bufs)
                nc.tensor.transpose(ps_t[:D, :H], af[:H, 0:D], ident[:H, :H])
                nc.vector.tensor_copy(a_tT[:], ps_t[:D, :H])
                # per-head bias column b[h] = a_t[h] . target  (and 0.2*b)
                ps_b = psp.tile([128, 128], F32, tag="pt", bufs=pt_bufs)
                nc.tensor.matmul(ps_b[:H, :1], a_tT[:, :H], tf[:, :1])
                nc.vector.tensor_copy(b_col[:], ps_b[:H, :1])
                nc.scalar.mul(b02_col[:], ps_b[:H, :1], 0.2)

            # ---------- persistent accumulators ----------
            acc_p = psp.tile([H, 4 * D], F32, tag="accP")  # sum_n w * block_j
            acc_s = psp.tile([H, 1], F32, tag="accS")      # sum_n w

            paths2d = paths_d.ap().rearrange("n l d -> n (l d)")

            # The per-tile work is split into head (DMA, DVE tree, PE
            # transposes, Act rT-copy) and tail (score matmuls, exps, max,
            # w-transposes, accumulation).  head(i) is emitted BEFORE
            # tail(i-1): engines execute their queues in strict FIFO order,
            # so emitting a tile's whole chain contiguously would chain the
            # next tile's rT-copy behind this tile's w-copy — a serial
            # cycle of ~6.9 us > the 5.8 us DMA period.  The head/tail
            # interleave keeps every engine's FIFO one iteration deep.
            def head(n0, nb):
                cnt = 128 * nb
                t = inp.tile([128, nb * FD], BF16, tag="in",
                             padded_shape=[128, 4 * FD])
                # partition p <- instances n0+nb*p+c, one contiguous line
                nc.sync.dma_start(
                    t[:],
                    paths2d[n0 : n0 + cnt, :].rearrange("(p b) f -> p (b f)", b=nb),
                )
                t3 = t.rearrange("p (b f) -> p b f", b=nb)
                # DVE tree: 16 -> 8 -> 4 node-blocks (bf16, 2 elem/cyc)
                t1 = work.tile([128, nb * 1024], BF16, tag="t1",
                               padded_shape=[128, 4 * 1024])
                t13 = t1.rearrange("p (b f) -> p b f", b=nb)
                nc.vector.tensor_add(t13[:, :, :], t3[:, :, 0:1024], t3[:, :, 1024:2048])
                t2 = work.tile([128, nb * 512], BF16, tag="t2", bufs=3,
                               padded_shape=[128, 4 * 512])
                t23 = t2.rearrange("p (b f) -> p b f", b=nb)
                nc.vector.tensor_add(t23[:, :, :], t13[:, :, 0:512], t13[:, :, 512:1024])

                # PE transposes of all 4 node-blocks -> [D, cnt] each.
                # 16-bit PSUM writes cannot accumulate on TRN2 (TRN3+ only),
                # so every transpose is an independent single write; the
                # 4-block fold happens in the fp32-accumulating e-matmuls.
                pt = psp.tile([128, nb * 512], BF16, tag="pt", bufs=pt_bufs,
                              padded_shape=[128, 2048])
                for j in range(4):
                    for c in range(nb):
                        nc.tensor.matmul(
                            pt[:, (j * nb + c) * 128 : (j * nb + c + 1) * 128],
                            t2[:, (c * 4 + j) * 128 : (c * 4 + j + 1) * 128],
                            identb[:, :],
                            is_transpose=True,
                            start=True, stop=True,
                            skip_group_check=True,
                        )
                rT = work.tile([128, nb * 512], BF16, tag="rT", bufs=3,
                               padded_shape=[128, 2048])
                # split the copy so the first e-matmuls can start while the
                # second half is still copying
                half = nb * 256
                nc.scalar.copy(rT[:, 0:half], pt[:, 0:half])
                nc.scalar.copy(rT[:, half : 2 * half], pt[:, half : 2 * half])
                return t2, rT, nb

            def tail(state, first, last):
                t2, rT, nb = state
                # scores eT[h, n] (pre-bias): accumulate the 4 block-slabs;
                # a_rT stays stationary across all four matmuls
                eT = psp.tile([H, nb * 128], F32, tag="eT", bufs=eT_bufs,
                              padded_shape=[H, 512])
                for j in range(4):
                    nc.tensor.matmul(
                        eT[:, :], a_rT[:, :],
                        rT[:, j * nb * 128 : (j + 1) * nb * 128],
                        start=(j == 0), stop=(j == 3),
                    )
                # w = max(exp(e + b), exp(0.2e + 0.2b)) = exp(leakyrelu(e + b))
                wa = work.tile([H, nb * 128], BF16, tag="wa",
                               padded_shape=[H, 512])
                nc.scalar.activation(wa[:], eT[:, :], AF.Exp, bias=b_col[:], scale=1.0)
                wb = work.tile([H, nb * 128], BF16, tag="wb",
                               padded_shape=[H, 512])
                nc.scalar.activation(wb[:], eT[:, :], AF.Exp, bias=b02_col[:], scale=0.2)
                w2 = work.tile([H, nb * 128], BF16, tag="w2",
                               padded_shape=[H, 512])
                nc.vector.tensor_max(w2[:], wa[:], wb[:])
                # transpose w back to instance-layout [cnt, H]
                # (independent single writes into distinct slices)
                wps = psp.tile([128, nb * H], BF16, tag="wps", bufs=wps_bufs,
                               padded_shape=[128, 4 * H])
                for c in range(nb):
                    nc.tensor.matmul(
                        wps[:, c * H : (c + 1) * H],
                        w2[:H, c * 128 : (c + 1) * 128],
                        identb[:H, :H],
                        is_transpose=True,
                        start=True, stop=True,
                        skip_group_check=True,
                    )
                w_sb = work.tile([128, nb * H], BF16, tag="w",
                                 padded_shape=[128, 4 * H])
                nc.scalar.copy(w_sb[:], wps[:])
                # accumulate sum_n w*block_j and sum_n w
                for c in range(nb):
                    bfirst = first and c == 0
                    blast = last and c == nb - 1
                    nc.tensor.matmul(
                        acc_p[:H, :],
                        w_sb[:, c * H : (c + 1) * H],
                        t2[:, c * 512 : (c + 1) * 512],
                        start=bfirst, stop=blast,
                    )
                    nc.tensor.matmul(
                        acc_s[:H, :],
                        w_sb[:, c * H : (c + 1) * H],
                        ones_col[:, :],
                        start=bfirst, stop=blast,
                    )

            # repeat>1 is a timing-only mode (re-streams the same shard)
            seq = []
            for _r in range(repeat):
                n0 = 0
                for cnt, nb in BIGS:
                    seq.append((n0, nb))
                    n0 += cnt
            pending = None
            for i, (n0, nb) in enumerate(seq):
                st = head(n0, nb)
                if i == 0:
                    setup_consts()
                if pending is not None:
                    tail(pending, first=(i == 1), last=False)
                pending = st
            tail(pending, first=(len(seq) == 1), last=True)

            # ---------- emit per-core partial [p_raw | s] ----------
            # cross-core combine + softmax normalization happens on host
            accs = work.tile([H, 4 * D], F32, tag="accs")
            nc.vector.tensor_copy(accs[:H, :], acc_p[:H, :])
            fold = work.tile([H, 2 * D], F32, tag="fold")
            nc.vector.tensor_add(fold[:H, :], accs[:H, 0 : 2 * D], accs[:H, 2 * D : 4 * D])
            part = work.tile([H, D + 1], F32, tag="part")
            nc.vector.tensor_add(part[:H, 0:D], fold[:H, 0:D], fold[:H, D : 2 * D])
            nc.vector.tensor_copy(part[:H, D : D + 1], acc_s[:H, :])
            nc.sync.dma_start(
                out_d.ap().rearrange("(h d) -> h d", d=D + 1), part[:]
            )

    nc.compile()
    return nc


def _make_in_maps(target_feat, paths, attn_fc):
    """Shard + zero-pad to NSP and cast the streamed tensor to bf16."""
    tgt = np.ascontiguousarray(np.asarray(target_feat, dtype=np.float32))
    af = np.ascontiguousarray(np.asarray(attn_fc, dtype=np.float32))
    shards = np.asarray(paths, dtype=np.float32).reshape(NCORES, NS, L, D)
    padded = np.zeros((NCORES, NSP, L, D), dtype=ml_dtypes.bfloat16)
    padded[:, :NS] = shards.astype(ml_dtypes.bfloat16)
    return [
        {"paths": padded[i], "target_feat": tgt, "attn_fc": af}
        for i in range(NCORES)
    ]


def kernel(target_feat, paths, attn_fc, **_unused):
    global _cached_nc
    if _cached_nc is None:
        _cached_nc = _build()
    nc = _cached_nc

    in_maps = _make_in_maps(target_feat, paths, attn_fc)
    res = run_bass_kernel_spmd(nc, in_maps, core_ids=list(range(NCORES)))
    # host-side combine of the 8 per-core partials [H, D+1]
    tot = np.zeros((H, D + 1), dtype=np.float64)
    for i in range(NCORES):
        tot += np.asarray(res.results[i]["out"], dtype=np.float64).reshape(
            H, D + 1
        )
    # subtract the pad instances' exact softmax-weight contribution:
    # each zero instance has reps=0 -> w = exp(leakyrelu(b_h))
    b = attn_fc[:, :D].astype(np.float64) @ np.asarray(target_feat, np.float64)
    w_pad = np.exp(np.where(b > 0, b, 0.2 * b))
    s = tot[:, D] - NCORES * NPAD * w_pad
    out = tot[:, :D] / (L * s[:, None])
    return np.ascontiguousarray(out.reshape(H * D).astype(np.float32))
